# revision 20
# baseline (speedup 1.0000x reference)
"""Trainium2 Bass kernel for nn_ContextBiasingLayer.

Math (per batch):
  q_proj = query @ Wq.T + bq                      [T, OD]
  q = gelu(q_proj); k = gelu(key @ Wk.T + bk); v = gelu(value @ Wv.T + bv)
  scores = (q_h @ k_h.T) / 8, masked softmax over C, attn @ v_h -> attn_out
  out = concat(LN(attn_out), LN(q_proj)) @ Wo.T + bo

Strategy: data-parallel over batch (2 batches per core, 8 cores).
Activations live in transposed [feature, token] layout so every matmul
contracts over partitions.  LayerNorm gains/biases are folded into Wo on the
host; the per-token mean correction rides the output matmul as an augmented
contraction row, and the rsqrt(var) factor is applied per-partition in a
fused scalar_tensor_tensor epilogue.  Softmax skips max-subtraction (scores
are O(1)), applies the mask multiplicatively in bf16, and defers the
denominator division to the [64, T] attn@v output (8x fewer elements).
"""

import os
import sys
from contextlib import ExitStack

import numpy as np

sys.path.insert(0, "/opt/trn_rl_repo")

import concourse.bass as bass
import concourse.tile as tile
from concourse import bacc, mybir
from concourse.bass_utils import run_bass_kernel_spmd

try:
    import ml_dtypes

    BF16 = ml_dtypes.bfloat16
except ImportError:  # pragma: no cover
    BF16 = np.float32

# Problem shape (hardcoded per contract)
B, T, C = 16, 1024, 256
QD = KD = OD = 512
H, DK = 8, 64
N_CORES = 8
BPC = B // N_CORES          # batches per core
TT = BPC * T                # query tokens per core  (2048)
CT = BPC * C                # kv tokens per core     (512)
LN_EPS = 1e-12

f32 = mybir.dt.float32
f32r = mybir.dt.float32r
bf16 = mybir.dt.bfloat16

_CACHED_NC = None
LAST_RESULT = None          # test.py reads exec_time_ns from here


def _build_nc():
    nc = bacc.Bacc("TRN2", target_bir_lowering=False, debug=False, num_devices=N_CORES)
    d_xq = nc.declare_dram_parameter("xq", [QD, TT], bf16, isOutput=False)
    d_xk = nc.declare_dram_parameter("xk", [KD, CT], bf16, isOutput=False)
    d_xv = nc.declare_dram_parameter("xv", [KD, CT], bf16, isOutput=False)
    d_m01 = nc.declare_dram_parameter("m01", [2, 128, TT], bf16, isOutput=False)
    d_wq = nc.declare_dram_parameter("wq", [QD, OD], bf16, isOutput=False)
    d_wk = nc.declare_dram_parameter("wk", [KD, OD], bf16, isOutput=False)
    d_wv = nc.declare_dram_parameter("wv", [KD + 1, OD], bf16, isOutput=False)
    d_woA = nc.declare_dram_parameter("woA", [OD + 1, OD], bf16, isOutput=False)
    d_woQ = nc.declare_dram_parameter("woQ", [QD + 1, OD], bf16, isOutput=False)
    d_bq = nc.declare_dram_parameter("bq", [128, 4], f32, isOutput=False)
    d_bk = nc.declare_dram_parameter("bk", [128, 4], f32, isOutput=False)
    d_bo = nc.declare_dram_parameter("bo", [128, OD], f32, isOutput=False)
    d_out = nc.declare_dram_parameter("out", [TT, OD], f32, isOutput=True)

    ACT = mybir.ActivationFunctionType
    ALU = mybir.AluOpType

    with tile.TileContext(nc) as tc, ExitStack() as ctx:
        consts = ctx.enter_context(tc.tile_pool(name="consts", bufs=1))
        persist = ctx.enter_context(tc.tile_pool(name="persist", bufs=1))
        small = ctx.enter_context(tc.tile_pool(name="small", bufs=1))
        dram = ctx.enter_context(tc.tile_pool(name="dram", bufs=1, space="DRAM"))
        dram4 = ctx.enter_context(tc.tile_pool(name="dram4", bufs=4, space="DRAM"))

        # ---- constants / weights (persistent across phases) ----
        woA_sb = consts.tile([128, 4, OD], bf16)
        nc.sync.dma_start(out=woA_sb, in_=d_woA[0:OD, :].rearrange("(c p) n -> p c n", p=128))
        woAa_sb = consts.tile([1, OD], bf16)
        nc.sync.dma_start(out=woAa_sb, in_=d_woA[OD : OD + 1, :])
        woQ_sb = consts.tile([128, 4, OD], bf16)
        nc.sync.dma_start(out=woQ_sb, in_=d_woQ[0:QD, :].rearrange("(c p) n -> p c n", p=128))
        woQa_sb = consts.tile([1, OD], bf16)
        nc.sync.dma_start(out=woQa_sb, in_=d_woQ[QD : QD + 1, :])
        bob_sb = consts.tile([128, OD], f32)
        nc.sync.dma_start(out=bob_sb, in_=d_bo[:])
        m01_sb = consts.tile([128, 2, TT], bf16)
        nc.sync.dma_start(out=m01_sb, in_=d_m01[:].rearrange("c p t -> p c t"))

        ones_col = consts.tile([128, 1], bf16)
        nc.vector.memset(ones_col, 1.0)
        ones_row = consts.tile([1, 128], bf16)
        nc.vector.memset(ones_row, 1.0)
        eps_sb = consts.tile([128, 1], f32)
        nc.vector.memset(eps_sb, LN_EPS)

        # ---- persistent activations ----
        qT_sb = persist.tile([128, 4, TT], bf16)     # gelu(q_proj)^T, scores rhs
        qraw_sb = persist.tile([128, 4, TT], bf16)    # q_proj^T (pre-gelu, +bq)
        kT_sb = persist.tile([128, 4, CT], bf16)     # gelu(k_proj)^T
        v_sb = persist.tile([128, 4, H, DK + 1], bf16)  # gelu(v) natural + ones col
        araw_sb = persist.tile([128, 4, TT], bf16)    # attn_out^T

        nc.vector.memset(v_sb[:, :, :, DK : DK + 1], 1.0)

        # stat rows: sum at partition 0 (matmul lhsT base must match rhs base 0)
        rowsA_sb = small.tile([33, TT], bf16)
        rowsQ_sb = small.tile([33, TT], bf16)
        sA_row = rowsA_sb[0:1, :]
        ssqA_row = rowsA_sb[32:33, :]
        sQ_row = rowsQ_sb[0:1, :]
        ssqQ_row = rowsQ_sb[32:33, :]
        sA_c = small.tile([128, 16], bf16)
        ssqA_c = small.tile([128, 16], bf16)
        sQ_c = small.tile([128, 16], bf16)
        ssqQ_c = small.tile([128, 16], bf16)
        rA_c = small.tile([128, 16], f32)
        rQ_c = small.tile([128, 16], f32)
        t1_c = small.tile([128, 16], f32)
        t2_c = small.tile([128, 16], f32)
        scratch_d = dram.tile([4, TT], bf16)
        tch_a = small.tile([1, 1], f32)
        tch_d = small.tile([1, 1], f32)

        def touch(*aps):
            """Tiny per-engine reads of DMA-fed tiles.  Each op makes the
            engine's vector clock observe that tile's DMA queue, so later hot
            instructions don't accumulate more sem waits than the ISA's
            per-instruction wait slots allow (walrus: "Too many sync wait
            commands")."""
            for ap in aps:
                nc.scalar.copy(tch_a, ap)
                nc.vector.tensor_copy(tch_d, ap)

        touch(
            woA_sb[0:1, 0, 0:1],
            woQ_sb[0:1, 0, 0:1],
            woAa_sb[0:1, 0:1],
            woQa_sb[0:1, 0:1],
            bob_sb[0:1, 0:1],
            m01_sb[0:1, 0, 0:1],
        )

        NT = TT // 512   # 4 big token chunks of 512

        # =========== Phase 1: projections ===========
        with tc.tile_pool(name="p1c", bufs=1) as p1c, \
             tc.tile_pool(name="xq", bufs=2) as xqp, \
             tc.tile_pool(name="pp_pj", bufs=4, space="PSUM") as pp_pj:
            wq_sb = p1c.tile([128, 4, OD], bf16)
            nc.sync.dma_start(out=wq_sb, in_=d_wq[:].rearrange("(c p) n -> p c n", p=128))
            wk_sb = p1c.tile([128, 4, OD], bf16)
            nc.sync.dma_start(out=wk_sb, in_=d_wk[:].rearrange("(c p) n -> p c n", p=128))
            wv_sb = p1c.tile([128, 4, OD], bf16)
            nc.sync.dma_start(out=wv_sb, in_=d_wv[0:KD, :].rearrange("(c p) n -> p c n", p=128))
            wvaug_sb = p1c.tile([1, OD], bf16)
            nc.sync.dma_start(out=wvaug_sb, in_=d_wv[KD : KD + 1, :])
            bq_sb = p1c.tile([128, 4], f32)
            nc.sync.dma_start(out=bq_sb, in_=d_bq[:])
            bk_sb = p1c.tile([128, 4], f32)
            nc.sync.dma_start(out=bk_sb, in_=d_bk[:])
            xk_sb = p1c.tile([128, 4, CT], bf16)
            nc.sync.dma_start(out=xk_sb, in_=d_xk[:].rearrange("(c p) t -> p c t", p=128))
            xv_sb = p1c.tile([128, 4, CT], bf16)
            nc.sync.dma_start(out=xv_sb, in_=d_xv[:].rearrange("(c p) t -> p c t", p=128))
            touch(
                wq_sb[0:1, 0, 0:1],
                wk_sb[0:1, 0, 0:1],
                wv_sb[0:1, 0, 0:1],
                wvaug_sb[0:1, 0:1],
                bq_sb[0:1, 0:1],
                bk_sb[0:1, 0:1],
                xk_sb[0:1, 0, 0:1],
                xv_sb[0:1, 0, 0:1],
            )

            # q projection: out [od, t] ; lhsT = wq chunk, rhs = xq chunk
            for tc_i in range(NT):
                sl = bass.ts(tc_i, 512)
                xq_t = xqp.tile([128, 4, 512], bf16)
                nc.sync.dma_start(
                    out=xq_t,
                    in_=d_xq[:].rearrange("(c p) t -> p c t", p=128)[:, :, sl],
                )
                touch(xq_t[0:1, 0, 0:1])
                for oc in range(4):
                    ps = pp_pj.tile([128, 512], f32)
                    for kc in range(4):
                        nc.tensor.matmul(
                            ps,
                            wq_sb[:, kc, bass.ts(oc, 128)],
                            xq_t[:, kc, :],
                            start=(kc == 0),
                            stop=(kc == 3),
                        )
                    nc.scalar.activation(
                        qT_sb[:, oc, sl], ps, ACT.Gelu,
                        bias=bq_sb[:, oc : oc + 1], scale=1.0,
                    )
                    nc.scalar.activation(
                        qraw_sb[:, oc, sl], ps, ACT.Identity,
                        bias=bq_sb[:, oc : oc + 1], scale=1.0,
                    )

            # k projection: out [od, c]
            for oc in range(4):
                ps = pp_pj.tile([128, 512], f32)
                for kc in range(4):
                    nc.tensor.matmul(
                        ps,
                        wk_sb[:, kc, bass.ts(oc, 128)],
                        xk_sb[:, kc, :],
                        start=(kc == 0),
                        stop=(kc == 3),
                    )
                nc.scalar.activation(
                    kT_sb[:, oc, :], ps, ACT.Gelu,
                    bias=bk_sb[:, oc : oc + 1], scale=1.0,
                )

            # v projection: out [c, od] natural; bias via augmented K row
            for cc in range(4):
                ps = pp_pj.tile([128, 512], f32)
                for kc in range(4):
                    nc.tensor.matmul(
                        ps,
                        xv_sb[:, kc, bass.ts(cc, 128)],
                        wv_sb[:, kc, :],
                        start=(kc == 0),
                        stop=False,
                    )
                nc.tensor.matmul(
                    ps,
                    ones_row,
                    wvaug_sb,
                    start=False,
                    stop=True,
                )
                nc.scalar.activation(
                    v_sb[:, cc, :, 0:DK], ps.rearrange("p (h d) -> p h d", h=H),
                    ACT.Gelu, scale=1.0,
                )

        # =========== Phase 2: attention ===========
        with tc.tile_pool(name="pp_sc", bufs=4, space="PSUM") as pp_sc, \
             tc.tile_pool(name="pp_u", bufs=2, space="PSUM") as pp_u, \
             tc.tile_pool(name="epool", bufs=3) as epool, \
             tc.tile_pool(name="sinvp", bufs=4) as sinvp:
            for b in range(BPC):
                for h in range(H):
                    po = (h % 2) * 64
                    oc = h // 2
                    for tc_i in range(2):      # two 512-token chunks per batch
                        tsl = bass.ds(b * T + tc_i * 512, 512)
                        e_t = epool.tile([128, 2, 512], bf16)
                        for cc in range(2):
                            ps = pp_sc.tile([128, 512], f32)
                            nc.tensor.matmul(
                                ps,
                                kT_sb[po : po + 64, oc, bass.ds(b * C + cc * 128, 128)],
                                qT_sb[po : po + 64, oc, tsl],
                                start=True,
                                stop=True,
                            )
                            # exp(scores/8), fp32 psum -> bf16 sbuf
                            nc.scalar.activation(
                                e_t[:, cc, :], ps, ACT.Exp, scale=0.125,
                            )
                        # multiplicative mask (bf16, 2x mode)
                        nc.vector.tensor_mul(
                            e_t[:, :, :],
                            e_t[:, :, :],
                            m01_sb[:, :, tsl],
                        )
                        # U = [v | 1]^T-style: psum rows 0:64 = attnout, row 64 = colsum
                        up = pp_u.tile([65, 512], f32)
                        for cc in range(2):
                            nc.tensor.matmul(
                                up,
                                v_sb[:, 2 * b + cc, h, :],
                                e_t[:, cc, :],
                                start=(cc == 0),
                                stop=(cc == 1),
                            )
                        sinv = sinvp.tile([1, 512], f32)
                        nc.vector.reciprocal(sinv, up[64:65, :])
                        sdram = dram4.tile([1, 512], f32, tag="sinv_dram")
                        nc.sync.dma_start(out=sdram, in_=sinv)
                        sinvb = sinvp.tile([64, 512], f32)
                        nc.sync.dma_start(
                            out=sinvb,
                            in_=bass.AP(
                                tensor=sdram.tensor,
                                offset=sdram.offset,
                                ap=[[0, 64]] + list(sdram.ap[1:]),
                            ),
                        )
                        nc.vector.tensor_mul(
                            araw_sb[po : po + 64, oc, tsl],
                            up[0:64, :],
                            sinvb,
                        )

        # =========== Phase 3: LN stats ===========
        with tc.tile_pool(name="pp_st", bufs=4, space="PSUM") as pp_st, \
             tc.tile_pool(name="sqp", bufs=3) as sqp:
            for src, s_row, ssq_row in (
                (araw_sb, sA_row, ssqA_row),
                (qraw_sb, sQ_row, ssqQ_row),
            ):
                for tc_i in range(NT):
                    sl = bass.ts(tc_i, 512)
                    ps_s = pp_st.tile([1, 512], f32)
                    ps_q = pp_st.tile([1, 512], f32)
                    for oc in range(4):
                        nc.tensor.matmul(
                            ps_s,
                            ones_col,
                            src[:, oc, sl],
                            start=(oc == 0),
                            stop=(oc == 3),
                        )
                    for oc in range(4):
                        sq_t = sqp.tile([128, 512], bf16)
                        nc.scalar.activation(sq_t, src[:, oc, sl], ACT.Square)
                        nc.tensor.matmul(
                            ps_q,
                            ones_col,
                            sq_t,
                            start=(oc == 0),
                            stop=(oc == 3),
                        )
                    nc.scalar.activation(s_row[:, sl], ps_s, ACT.Copy, scale=1.0)
                    nc.scalar.activation(ssq_row[:, sl], ps_q, ACT.Copy, scale=1.0)

            # transpose stat rows to [128, 16] column layout via DRAM bounce
            for i, (row, col) in enumerate(
                ((sA_row, sA_c), (ssqA_row, ssqA_c), (sQ_row, sQ_c), (ssqQ_row, ssqQ_c))
            ):
                nc.sync.dma_start(out=scratch_d[i : i + 1, :], in_=row[:])
                nc.sync.dma_start(
                    out=col,
                    in_=scratch_d[i : i + 1, :].rearrange("a (n p) -> (a p) n", p=128),
                )

            # r = rsqrt(var + eps), var = (ssq - s^2/512)/512
            for s_c, ssq_c, r_c in ((sA_c, ssqA_c, rA_c), (sQ_c, ssqQ_c, rQ_c)):
                nc.vector.tensor_mul(t1_c, s_c, s_c)
                nc.vector.scalar_tensor_tensor(
                    out=t2_c, in0=t1_c, scalar=-1.0 / OD, in1=ssq_c,
                    op0=ALU.mult, op1=ALU.add,
                )
                nc.scalar.activation(
                    t1_c, t2_c, ACT.Sqrt, bias=eps_sb, scale=1.0 / OD,
                )
                nc.vector.reciprocal(r_c, t1_c)

        # =========== Phase 4: output projection ===========
        with tc.tile_pool(name="pp_fin", bufs=4, space="PSUM") as pp_fin, \
             tc.tile_pool(name="outp", bufs=3) as outp, \
             tc.tile_pool(name="tmpp", bufs=3) as tmpp:
            for tc_i in range(16):          # 128-token chunks
                sl = bass.ts(tc_i, 128)
                ps_a = pp_fin.tile([128, 512], f32)
                ps_q = pp_fin.tile([128, 512], f32)
                for kc in range(4):
                    nc.tensor.matmul(
                        ps_a,
                        araw_sb[:, kc, sl],
                        woA_sb[:, kc, :],
                        start=(kc == 0),
                        stop=False,
                    )
                nc.tensor.matmul(
                    ps_a,
                    sA_row[:, sl],
                    woAa_sb,
                    start=False,
                    stop=True,
                )
                for kc in range(4):
                    nc.tensor.matmul(
                        ps_q,
                        qraw_sb[:, kc, sl],
                        woQ_sb[:, kc, :],
                        start=(kc == 0),
                        stop=False,
                    )
                nc.tensor.matmul(
                    ps_q,
                    sQ_row[:, sl],
                    woQa_sb,
                    start=False,
                    stop=True,
                )
                tmp_t = tmpp.tile([128, 512], f32)
                nc.vector.scalar_tensor_tensor(
                    out=tmp_t, in0=ps_q, scalar=rQ_c[:, tc_i : tc_i + 1], in1=bob_sb,
                    op0=ALU.mult, op1=ALU.add,
                )
                out_t = outp.tile([128, 512], f32)
                nc.vector.scalar_tensor_tensor(
                    out=out_t, in0=ps_a, scalar=rA_c[:, tc_i : tc_i + 1], in1=tmp_t,
                    op0=ALU.mult, op1=ALU.add,
                )
                nc.sync.dma_start(out=d_out[sl, :], in_=out_t)

    return nc


def kernel(**inputs):
    global _CACHED_NC, LAST_RESULT
    q = np.asarray(inputs["query"], np.float32)
    k = np.asarray(inputs["key"], np.float32)
    v = np.asarray(inputs["value"], np.float32)
    mask = np.asarray(inputs["mask"])
    Wq = np.asarray(inputs["Wq"], np.float32)
    bq = np.asarray(inputs["bq"], np.float32)
    Wk = np.asarray(inputs["Wk"], np.float32)
    bk = np.asarray(inputs["bk"], np.float32)
    Wv = np.asarray(inputs["Wv"], np.float32)
    bv = np.asarray(inputs["bv"], np.float32)
    q_ln_g = np.asarray(inputs["q_ln_g"], np.float32)
    q_ln_b = np.asarray(inputs["q_ln_b"], np.float32)
    attn_ln_g = np.asarray(inputs["attn_ln_g"], np.float32)
    attn_ln_b = np.asarray(inputs["attn_ln_b"], np.float32)
    Wo = np.asarray(inputs["Wo"], np.float32)
    bo = np.asarray(inputs["bo"], np.float32)

    # fold LN affine params into the output projection
    g_cat = np.concatenate([attn_ln_g, q_ln_g])
    b_cat = np.concatenate([attn_ln_b, q_ln_b])
    Wop = (Wo * g_cat[None, :]).astype(np.float32)          # [OD, OD+QD]
    bop = (bo + Wo @ b_cat).astype(np.float32)
    woA = np.ascontiguousarray(
        np.concatenate([Wop[:, :OD].T, (-Wop[:, :OD].sum(1) / OD)[None, :]], 0)
    ).astype(BF16)
    woQ = np.ascontiguousarray(
        np.concatenate([Wop[:, OD:].T, (-Wop[:, OD:].sum(1) / OD)[None, :]], 0)
    ).astype(BF16)
    wqT = np.ascontiguousarray(Wq.T).astype(BF16)
    wkT = np.ascontiguousarray(Wk.T).astype(BF16)
    wvT_aug = np.ascontiguousarray(
        np.concatenate([Wv.T, bv[None, :]], 0)
    ).astype(BF16)
    bq4 = np.ascontiguousarray(bq.reshape(4, 128).T, np.float32)
    bk4 = np.ascontiguousarray(bk.reshape(4, 128).T, np.float32)
    bob = np.ascontiguousarray(np.broadcast_to(bop[None, :], (128, OD)), np.float32)

    if _CACHED_NC is None:
        nc = _build_nc()
        if not nc.is_finalized():
            nc.finalize()
        _CACHED_NC = nc
    nc = _CACHED_NC

    in_maps = []
    for core in range(N_CORES):
        b0 = core * BPC
        xqT = np.ascontiguousarray(
            q[b0 : b0 + BPC].transpose(2, 0, 1).reshape(QD, TT)
        ).astype(BF16)
        xkT = np.ascontiguousarray(
            k[b0 : b0 + BPC].transpose(2, 0, 1).reshape(KD, CT)
        ).astype(BF16)
        xvT = np.ascontiguousarray(
            v[b0 : b0 + BPC].transpose(2, 0, 1).reshape(KD, CT)
        ).astype(BF16)
        m01 = np.ascontiguousarray(
            (mask[b0 : b0 + BPC] != 0)
            .transpose(2, 0, 1)
            .reshape(2, 128, TT)
        ).astype(BF16)
        in_maps.append(
            {
                "xq": xqT,
                "xk": xkT,
                "xv": xvT,
                "m01": m01,
                "wq": wqT,
                "wk": wkT,
                "wv": wvT_aug,
                "woA": woA,
                "woQ": woQ,
                "bq": bq4,
                "bk": bk4,
                "bo": bob,
            }
        )

    trace = bool(int(os.environ.get("KERNEL_TRACE", "0")))
    res = run_bass_kernel_spmd(
        nc, in_maps, list(range(N_CORES)), trace=trace,
    )
    LAST_RESULT = res
    out = np.concatenate(
        [res.results[i]["out"].reshape(BPC, T, OD) for i in range(N_CORES)], 0
    )
    return out.astype(np.float32)
